# revision 2
# baseline (speedup 1.0000x reference)
"""Trainium2 Bass kernel for the ComplexSSM problem.

Math (per batch b, channel r):
    lam = -5*sigmoid(lambda_raw); mag = exp(lam); a = mag*exp(i*omega)
    x[t] = W_proj @ u[t]                       (real)
    h[t] = a*h[t-1] + x[t],  h[-1] = h0        (complex, diagonal)
    y[t] = concat(h_r[t], h_i[t]) + W_res @ u[t]
    out  = layernorm(y) * gamma + beta
    finals = h[T-1]

Polar decomposition of the scan (the key trick):
    h[t] = exp(i*omega*(t+1)) * g[t]
    g[t] = mag*g[t-1] + exp(-i*omega*(t+1))*x[t],   g[-1] = h0
so with C[t]=cos(omega*(t+1)), S[t]=sin(omega*(t+1)) (host fp64 tables):
    g_r = scan(mag, x*C, +),  g_i = scan(mag, -(x*S), i.e. op1=subtract)
    h_r = C*g_r - S*g_i,      h_i = C*g_i + S*g_r
Each scan is a real first-order recurrence with constant per-partition
coefficient -> native DVE tensor_tensor_scan.

Sharding: batch (B=8) -> one batch per NeuronCore, 8 cores.
"""

import sys

if "/opt/trn_rl_repo" not in sys.path:
    sys.path.insert(0, "/opt/trn_rl_repo")

import math

import numpy as np

import concourse.bacc as bacc
import concourse.mybir as mybir
import concourse.tile as tile
from concourse.bass_utils import run_bass_kernel_spmd

# Problem constants (hardcoded per the contract)
B, T, D, R = 8, 4096, 1024, 512
CH = 512          # time chunk (scan segment length)
NCH = T // CH     # 8 chunks
RT = R // 128     # 4 channel tiles
DT = D // 128     # 8 contraction tiles
SUB = CH // 128   # 4 psum output subtiles per chunk
LN_EPS = 1e-5

F32 = mybir.dt.float32
F32R = mybir.dt.float32r
AX = mybir.AluOpType
AF = mybir.ActivationFunctionType

_CACHE = {}


def _mm(x, dtype):
    """View an fp32 AP as the matmul dtype (float32r for fast PE path)."""
    if dtype is F32:
        return x
    return x.bitcast(dtype)


def build_program(mm_dtype=F32, apply_gamma_beta=False):
    nc = bacc.Bacc("TRN2", target_bir_lowering=False, debug=False, num_devices=B)

    u_b = nc.dram_tensor("u_b", [T, D], F32, kind="ExternalInput").ap()
    h0r = nc.dram_tensor("h0r", [R, 1], F32, kind="ExternalInput").ap()
    h0i = nc.dram_tensor("h0i", [R, 1], F32, kind="ExternalInput").ap()
    wpt = nc.dram_tensor("wpt", [D, R], F32, kind="ExternalInput").ap()      # W_proj.T
    wrt = nc.dram_tensor("wrt", [D, 2 * R], F32, kind="ExternalInput").ap()  # W_res.T
    ctab = nc.dram_tensor("ctab", [R, T], F32, kind="ExternalInput").ap()
    stab = nc.dram_tensor("stab", [R, T], F32, kind="ExternalInput").ap()
    magb = nc.dram_tensor("magb", [R, CH], F32, kind="ExternalInput").ap()
    idn = nc.dram_tensor("idn", [128, 128], F32, kind="ExternalInput").ap()
    gmb = nc.dram_tensor("gmb", [128, 2 * R], F32, kind="ExternalInput").ap()
    btb = nc.dram_tensor("btb", [128, 2 * R], F32, kind="ExternalInput").ap()

    out_b = nc.dram_tensor("out_b", [T, 2 * R], F32, kind="ExternalOutput").ap()
    fin_r = nc.dram_tensor("fin_r", [R, 1], F32, kind="ExternalOutput").ap()
    fin_i = nc.dram_tensor("fin_i", [R, 1], F32, kind="ExternalOutput").ap()

    with tile.TileContext(nc) as tc:
        with tc.tile_pool(name="const", bufs=1) as cp, \
             tc.tile_pool(name="work", bufs=1) as wp, \
             tc.tile_pool(name="small", bufs=4) as sp, \
             tc.tile_pool(name="pt", bufs=2, space="PSUM") as ptp, \
             tc.tile_pool(name="px", bufs=2, space="PSUM") as pxp, \
             tc.tile_pool(name="py", bufs=2, space="PSUM") as pyp:

            # ---- static loads ----
            ident = cp.tile([128, 128], F32, name="ident", tag="ident")
            nc.sync.dma_start(out=ident, in_=idn)
            wpt_t = []
            wrt_t = []
            for d in range(DT):
                w1 = cp.tile([128, R], F32, name=f"wpt{d}", tag=f"wpt{d}")
                nc.sync.dma_start(out=w1, in_=wpt[d * 128:(d + 1) * 128, :])
                wpt_t.append(w1)
                w2 = cp.tile([128, 2 * R], F32, name=f"wrt{d}", tag=f"wrt{d}")
                nc.sync.dma_start(out=w2, in_=wrt[d * 128:(d + 1) * 128, :])
                wrt_t.append(w2)
            mag_t = []
            h0r_t = []
            h0i_t = []
            for g in range(RT):
                m = cp.tile([128, CH], F32, name=f"mag{g}", tag=f"mag{g}")
                nc.sync.dma_start(out=m, in_=magb[g * 128:(g + 1) * 128, :])
                mag_t.append(m)
                hr0 = cp.tile([128, 1], F32, name=f"h0r{g}", tag=f"h0r{g}")
                nc.sync.dma_start(out=hr0, in_=h0r[g * 128:(g + 1) * 128, :])
                h0r_t.append(hr0)
                hi0 = cp.tile([128, 1], F32, name=f"h0i{g}", tag=f"h0i{g}")
                nc.sync.dma_start(out=hi0, in_=h0i[g * 128:(g + 1) * 128, :])
                h0i_t.append(hi0)
            if apply_gamma_beta:
                gam = cp.tile([128, 2 * R], F32, name="gam", tag="gam")
                nc.sync.dma_start(out=gam, in_=gmb)
                bet = cp.tile([128, 2 * R], F32, name="bet", tag="bet")
                nc.sync.dma_start(out=bet, in_=btb)

            carry_r = h0r_t
            carry_i = h0i_t
            hr_last = None
            hi_last = None

            for c in range(NCH):
                # ---- load u rows for this chunk ----
                uraw = []
                for s in range(SUB):
                    ur = wp.tile([128, D], F32, name=f"uraw{s}", tag=f"uraw{s}",
                                 bufs=1)
                    r0 = c * CH + s * 128
                    nc.sync.dma_start(out=ur, in_=u_b[r0:r0 + 128, :])
                    uraw.append(ur)

                # ---- transpose u: uT[d] = [128d, CH_t] ----
                uT = []
                for d in range(DT):
                    pt = ptp.tile([128, CH], F32, name=f"pt{d}", tag="pt")
                    for s in range(SUB):
                        nc.tensor.matmul(
                            pt[:, s * 128:(s + 1) * 128],
                            lhsT=uraw[s][:, d * 128:(d + 1) * 128],
                            rhs=ident, is_transpose=True,
                            start=True, stop=True)
                    ut = wp.tile([128, CH], F32, name=f"uT{d}", tag=f"uT{d}",
                                 bufs=1)
                    nc.scalar.copy(ut, pt)
                    uT.append(ut)

                # ---- rotation tables for this chunk ----
                C_t = []
                S_t = []
                for g in range(RT):
                    ct = wp.tile([128, CH], F32, name=f"C{g}", tag=f"C{g}", bufs=2)
                    nc.sync.dma_start(
                        out=ct, in_=ctab[g * 128:(g + 1) * 128, c * CH:(c + 1) * CH])
                    C_t.append(ct)
                    st = wp.tile([128, CH], F32, name=f"S{g}", tag=f"S{g}", bufs=2)
                    nc.sync.dma_start(
                        out=st, in_=stab[g * 128:(g + 1) * 128, c * CH:(c + 1) * CH])
                    S_t.append(st)

                # ---- x projection + pre-rotation + scans + post-rotation ----
                hr_t = []
                hi_t = []
                new_carry_r = []
                new_carry_i = []
                for g in range(RT):
                    px = pxp.tile([128, CH], F32, name=f"px{g}", tag="px")
                    for d in range(DT):
                        nc.tensor.matmul(
                            px,
                            lhsT=_mm(wpt_t[d][:, g * 128:(g + 1) * 128], mm_dtype),
                            rhs=_mm(uT[d], mm_dtype),
                            start=(d == 0), stop=(d == DT - 1))
                    xtr = wp.tile([128, CH], F32, name=f"xtr{g}", tag=f"xtr{g}",
                                  bufs=1)
                    nc.vector.tensor_tensor(xtr, px, C_t[g], op=AX.mult)
                    xti = wp.tile([128, CH], F32, name=f"xti{g}", tag=f"xti{g}",
                                  bufs=1)
                    nc.vector.tensor_tensor(xti, px, S_t[g], op=AX.mult)

                    gr = wp.tile([128, CH], F32, name=f"gr{g}", tag=f"gr{g}", bufs=1)
                    nc.vector.tensor_tensor_scan(
                        gr, mag_t[g], xtr, carry_r[g], op0=AX.mult, op1=AX.add)
                    gi = wp.tile([128, CH], F32, name=f"gi{g}", tag=f"gi{g}", bufs=1)
                    nc.vector.tensor_tensor_scan(
                        gi, mag_t[g], xti, carry_i[g], op0=AX.mult, op1=AX.subtract)

                    ncr = sp.tile([128, 1], F32, name=f"ncr{g}", tag=f"ncr{g}",
                                  bufs=2)
                    nc.vector.tensor_copy(ncr, gr[:, CH - 1:CH])
                    new_carry_r.append(ncr)
                    nci = sp.tile([128, 1], F32, name=f"nci{g}", tag=f"nci{g}",
                                  bufs=2)
                    nc.vector.tensor_copy(nci, gi[:, CH - 1:CH])
                    new_carry_i.append(nci)

                    # h_r = C*g_r - S*g_i ; h_i = C*g_i + S*g_r
                    ta = wp.tile([128, CH], F32, name="ta", tag="ta", bufs=2)
                    nc.vector.tensor_tensor(ta, C_t[g], gr, op=AX.mult)
                    tb = wp.tile([128, CH], F32, name="tb", tag="tb", bufs=2)
                    nc.gpsimd.tensor_tensor(tb, S_t[g], gi, op=AX.mult)
                    hr = wp.tile([128, CH], F32, name=f"hr{g}", tag=f"hr{g}", bufs=1)
                    nc.vector.tensor_tensor(hr, ta, tb, op=AX.subtract)
                    tc2 = wp.tile([128, CH], F32, name="tc2", tag="tc2", bufs=2)
                    nc.gpsimd.tensor_tensor(tc2, C_t[g], gi, op=AX.mult)
                    td = wp.tile([128, CH], F32, name="td", tag="td", bufs=2)
                    nc.vector.tensor_tensor(td, S_t[g], gr, op=AX.mult)
                    hi = wp.tile([128, CH], F32, name=f"hi{g}", tag=f"hi{g}", bufs=1)
                    nc.gpsimd.tensor_tensor(hi, tc2, td, op=AX.add)
                    hr_t.append(hr)
                    hi_t.append(hi)

                carry_r = new_carry_r
                carry_i = new_carry_i

                # ---- residual matmul + h transpose + layernorm per subtile ----
                for s in range(SUB):
                    py = pyp.tile([128, 2 * R], F32, name="py", tag="py")
                    for d in range(DT):
                        nc.tensor.matmul(
                            py[:, 0:512],
                            lhsT=_mm(uT[d][:, s * 128:(s + 1) * 128], mm_dtype),
                            rhs=_mm(wrt_t[d][:, 0:512], mm_dtype),
                            start=(d == 0), stop=False)
                        nc.tensor.matmul(
                            py[:, 512:1024],
                            lhsT=_mm(uT[d][:, s * 128:(s + 1) * 128], mm_dtype),
                            rhs=_mm(wrt_t[d][:, 512:1024], mm_dtype),
                            start=(d == 0), stop=False)
                    for g in range(RT):
                        nc.tensor.matmul(
                            py[:, g * 128:(g + 1) * 128],
                            lhsT=hr_t[g][:, s * 128:(s + 1) * 128],
                            rhs=ident, is_transpose=True,
                            start=False, stop=True)
                        nc.tensor.matmul(
                            py[:, 512 + g * 128:512 + (g + 1) * 128],
                            lhsT=hi_t[g][:, s * 128:(s + 1) * 128],
                            rhs=ident, is_transpose=True,
                            start=False, stop=True)

                    # layernorm over the 1024 features
                    stats = sp.tile([128, 12], F32, name="stats", tag="stats")
                    nc.vector.bn_stats(stats[:, 0:6], py[:, 0:512])
                    nc.vector.bn_stats(stats[:, 6:12], py[:, 512:1024])
                    mv = sp.tile([128, 2], F32, name="mv", tag="mv")
                    nc.vector.bn_aggr(mv, stats)
                    varep = sp.tile([128, 1], F32, name="varep", tag="varep")
                    nc.vector.tensor_scalar_add(varep, mv[:, 1:2], LN_EPS)
                    rec = sp.tile([128, 1], F32, name="rec", tag="rec")
                    nc.vector.reciprocal(rec, varep)
                    rstd = sp.tile([128, 1], F32, name="rstd", tag="rstd")
                    nc.scalar.sqrt(rstd, rec)
                    nbias = sp.tile([128, 1], F32, name="nbias", tag="nbias")
                    nc.vector.scalar_tensor_tensor(
                        nbias, mv[:, 0:1], -1.0, rstd, op0=AX.mult, op1=AX.mult)
                    osb = wp.tile([128, 2 * R], F32, name="osb", tag="osb", bufs=2)
                    nc.scalar.activation(osb, py, AF.Identity, bias=nbias,
                                         scale=rstd)
                    if apply_gamma_beta:
                        nc.vector.tensor_tensor(osb, osb, gam, op=AX.mult)
                        nc.gpsimd.tensor_tensor(osb, osb, bet, op=AX.add)
                    r0 = c * CH + s * 128
                    nc.sync.dma_start(out=out_b[r0:r0 + 128, :], in_=osb)

                if c == NCH - 1:
                    hr_last = hr_t
                    hi_last = hi_t

            # ---- final state outputs ----
            for g in range(RT):
                nc.sync.dma_start(out=fin_r[g * 128:(g + 1) * 128, :],
                                  in_=hr_last[g][:, CH - 1:CH])
                nc.sync.dma_start(out=fin_i[g * 128:(g + 1) * 128, :],
                                  in_=hi_last[g][:, CH - 1:CH])

    nc.compile()
    return nc


def _prep_host(u, h0_r, h0_i, lambda_raw, omega, W_proj, W_res, ln_gamma, ln_beta):
    lam = -5.0 / (1.0 + np.exp(-lambda_raw.astype(np.float64)))
    mag = np.exp(lam).astype(np.float32)

    t_idx = np.arange(1, T + 1, dtype=np.float64)
    ang = omega.astype(np.float64)[:, None] * t_idx[None, :]
    ctab = np.cos(ang).astype(np.float32)
    stab = np.sin(ang).astype(np.float32)

    magb = np.ascontiguousarray(np.broadcast_to(mag[:, None], (R, CH)))
    wpt = np.ascontiguousarray(W_proj.T)
    wrt = np.ascontiguousarray(W_res.T)
    idn = np.eye(128, dtype=np.float32)
    gmb = np.ascontiguousarray(
        np.broadcast_to(ln_gamma[None, :], (128, 2 * R))).astype(np.float32)
    btb = np.ascontiguousarray(
        np.broadcast_to(ln_beta[None, :], (128, 2 * R))).astype(np.float32)

    shared = dict(wpt=wpt, wrt=wrt, ctab=ctab, stab=stab, magb=magb, idn=idn,
                  gmb=gmb, btb=btb)
    in_maps = []
    for b in range(B):
        m = dict(shared)
        m["u_b"] = np.ascontiguousarray(u[b])
        m["h0r"] = np.ascontiguousarray(h0_r[b][:, None])
        m["h0i"] = np.ascontiguousarray(h0_i[b][:, None])
        in_maps.append(m)
    return in_maps


def kernel(u, h0_r, h0_i, lambda_raw, omega, W_proj, W_res, ln_gamma, ln_beta,
           mm_dtype=None, trace=False):
    mm_dtype = F32R if mm_dtype is None else mm_dtype
    apply_gb = not (np.all(ln_gamma == 1.0) and np.all(ln_beta == 0.0))
    key = (str(mm_dtype), apply_gb)
    if key not in _CACHE:
        _CACHE[key] = build_program(mm_dtype=mm_dtype, apply_gamma_beta=apply_gb)
    nc = _CACHE[key]

    in_maps = _prep_host(u, h0_r, h0_i, lambda_raw, omega, W_proj, W_res,
                         ln_gamma, ln_beta)
    res = run_bass_kernel_spmd(nc, in_maps, core_ids=list(range(B)), trace=trace)

    out = np.stack([res.results[b]["out_b"] for b in range(B)])
    final_r = np.stack([res.results[b]["fin_r"][:, 0] for b in range(B)])
    final_i = np.stack([res.results[b]["fin_i"][:, 0] for b in range(B)])
    kernel.last_results = res
    return out, final_r, final_i


# revision 7
# speedup vs baseline: 1.5715x; 1.5715x over previous
"""Trainium2 Bass kernel for the ComplexSSM problem.

Math (per batch b, channel r):
    lam = -5*sigmoid(lambda_raw); mag = exp(lam); a = mag*exp(i*omega)
    x[t] = W_proj @ u[t]                       (real)
    h[t] = a*h[t-1] + x[t],  h[-1] = h0        (complex, diagonal)
    y[t] = concat(h_r[t], h_i[t]) + W_res @ u[t]
    out  = layernorm(y) * gamma + beta
    finals = h[T-1]

Polar decomposition of the scan (the key trick):
    h[t] = exp(i*omega*(t+1)) * g[t]
    g[t] = mag*g[t-1] + exp(-i*omega*(t+1))*x[t],   g[-1] = h0
so with C[t]=cos(omega*(t+1)), S[t]=sin(omega*(t+1)) (host fp64 tables):
    g_r = scan(mag, x*C, +),  g_i = scan(mag, -(x*S), i.e. op1=subtract)
    h_r = C*g_r - S*g_i,      h_i = C*g_i + S*g_r
Each scan is a real first-order recurrence with constant per-partition
coefficient -> native DVE tensor_tensor_scan.

Sharding: batch (B=8) -> one batch per NeuronCore, 8 cores.
"""

import sys

if "/opt/trn_rl_repo" not in sys.path:
    sys.path.insert(0, "/opt/trn_rl_repo")

import math

import numpy as np

import concourse.bacc as bacc
import concourse.mybir as mybir
import concourse.tile as tile
from concourse.bass_utils import run_bass_kernel_spmd

# Problem constants (hardcoded per the contract)
B, T, D, R = 8, 4096, 1024, 512
CH = 512          # time chunk (scan segment length)
NCH = T // CH     # 8 chunks
RT = R // 128     # 4 channel tiles
DT = D // 128     # 8 contraction tiles
SUB = CH // 128   # 4 psum output subtiles per chunk
LN_EPS = 1e-5

F32 = mybir.dt.float32
F32R = mybir.dt.float32r
AX = mybir.AluOpType
AF = mybir.ActivationFunctionType

_CACHE = {}


def build_program(mm_dtype=F32, apply_gamma_beta=False):
    nc = bacc.Bacc("TRN2", target_bir_lowering=False, debug=False, num_devices=B)

    u_b = nc.dram_tensor("u_b", [T, D], F32, kind="ExternalInput").ap()
    h0r = nc.dram_tensor("h0r", [R, 1], F32, kind="ExternalInput").ap()
    h0i = nc.dram_tensor("h0i", [R, 1], F32, kind="ExternalInput").ap()
    wpt = nc.dram_tensor("wpt", [D, R], F32, kind="ExternalInput").ap()      # W_proj.T
    wrt = nc.dram_tensor("wrt", [D, 2 * R], F32, kind="ExternalInput").ap()  # W_res.T
    ctab = nc.dram_tensor("ctab", [R, T], F32, kind="ExternalInput").ap()
    stab = nc.dram_tensor("stab", [R, T], F32, kind="ExternalInput").ap()
    magb = nc.dram_tensor("magb", [R, CH], F32, kind="ExternalInput").ap()
    idn = nc.dram_tensor("idn", [128, 128], F32, kind="ExternalInput").ap()
    gmb = nc.dram_tensor("gmb", [128, 2 * R], F32, kind="ExternalInput").ap()
    btb = nc.dram_tensor("btb", [128, 2 * R], F32, kind="ExternalInput").ap()

    out_b = nc.dram_tensor("out_b", [T, 2 * R], F32, kind="ExternalOutput").ap()
    fin_r = nc.dram_tensor("fin_r", [R, 1], F32, kind="ExternalOutput").ap()
    fin_i = nc.dram_tensor("fin_i", [R, 1], F32, kind="ExternalOutput").ap()

    with tile.TileContext(nc) as tc:
        with tc.tile_pool(name="const", bufs=1) as cp, \
             tc.tile_pool(name="work", bufs=1) as wp, \
             tc.tile_pool(name="small", bufs=4) as sp, \
             tc.tile_pool(name="pt", bufs=2, space="PSUM") as ptp, \
             tc.tile_pool(name="px", bufs=2, space="PSUM") as pxp, \
             tc.tile_pool(name="py", bufs=2, space="PSUM") as pyp:

            # ---- static loads ----
            # Matmul operands must be materialized as mm_dtype (the BIR
            # verifier requires explicit rounding for float32r inputs).
            WDT = mm_dtype
            ident = cp.tile([128, 128], F32, name="ident", tag="ident")
            nc.sync.dma_start(out=ident, in_=idn)
            wpt_t = []
            wrt_t = []
            for d in range(DT):
                w1 = cp.tile([128, R], WDT, name=f"wpt{d}", tag=f"wpt{d}")
                w2 = cp.tile([128, 2 * R], WDT, name=f"wrt{d}", tag=f"wrt{d}")
                if WDT is F32:
                    nc.sync.dma_start(out=w1, in_=wpt[d * 128:(d + 1) * 128, :])
                    nc.sync.dma_start(out=w2, in_=wrt[d * 128:(d + 1) * 128, :])
                else:
                    ws1 = wp.tile([128, R], F32, name="wstage1", tag="wstage1",
                                  bufs=2)
                    nc.sync.dma_start(out=ws1, in_=wpt[d * 128:(d + 1) * 128, :])
                    nc.scalar.copy(w1, ws1)
                    ws2 = wp.tile([128, 2 * R], F32, name="wstage2", tag="wstage2",
                                  bufs=2)
                    nc.sync.dma_start(out=ws2, in_=wrt[d * 128:(d + 1) * 128, :])
                    nc.scalar.copy(w2, ws2)
                wpt_t.append(w1)
                wrt_t.append(w2)
            mag_t = []
            h0r_t = []
            h0i_t = []
            for g in range(RT):
                m = cp.tile([128, CH], F32, name=f"mag{g}", tag=f"mag{g}")
                nc.sync.dma_start(out=m, in_=magb[g * 128:(g + 1) * 128, :])
                mag_t.append(m)
                hr0 = cp.tile([128, 1], F32, name=f"h0r{g}", tag=f"h0r{g}")
                nc.sync.dma_start(out=hr0, in_=h0r[g * 128:(g + 1) * 128, :])
                h0r_t.append(hr0)
                hi0 = cp.tile([128, 1], F32, name=f"h0i{g}", tag=f"h0i{g}")
                nc.sync.dma_start(out=hi0, in_=h0i[g * 128:(g + 1) * 128, :])
                h0i_t.append(hi0)
            if apply_gamma_beta:
                gam = cp.tile([128, 2 * R], F32, name="gam", tag="gam")
                nc.sync.dma_start(out=gam, in_=gmb)
                bet = cp.tile([128, 2 * R], F32, name="bet", tag="bet")
                nc.sync.dma_start(out=bet, in_=btb)

            carry_r = h0r_t
            carry_i = h0i_t
            hr_last = None
            hi_last = None

            for c in range(NCH):
                # ---- load u rows for this chunk ----
                uraw = []
                for s in range(SUB):
                    ur = wp.tile([128, D], F32, name=f"uraw{s}", tag=f"uraw{s}",
                                 bufs=1)
                    r0 = c * CH + s * 128
                    nc.sync.dma_start(out=ur, in_=u_b[r0:r0 + 128, :])
                    uraw.append(ur)

                # ---- transpose u: uT[d] = [128d, CH_t] ----
                uT = []
                for d in range(DT):
                    pt = ptp.tile([128, CH], F32, name=f"pt{d}", tag="pt")
                    for s in range(SUB):
                        nc.tensor.matmul(
                            pt[:, s * 128:(s + 1) * 128],
                            lhsT=uraw[s][:, d * 128:(d + 1) * 128],
                            rhs=ident, is_transpose=True,
                            start=True, stop=True)
                    ut = wp.tile([128, CH], WDT, name=f"uT{d}", tag=f"uT{d}",
                                 bufs=1)
                    nc.scalar.copy(ut, pt)
                    uT.append(ut)

                # ---- rotation tables for this chunk ----
                C_t = []
                S_t = []
                for g in range(RT):
                    ct = wp.tile([128, CH], F32, name=f"C{g}", tag=f"C{g}", bufs=2)
                    nc.sync.dma_start(
                        out=ct, in_=ctab[g * 128:(g + 1) * 128, c * CH:(c + 1) * CH])
                    C_t.append(ct)
                    st = wp.tile([128, CH], F32, name=f"S{g}", tag=f"S{g}", bufs=2)
                    nc.sync.dma_start(
                        out=st, in_=stab[g * 128:(g + 1) * 128, c * CH:(c + 1) * CH])
                    S_t.append(st)

                # ---- x projection + pre-rotation + scans + post-rotation ----
                hr_t = []
                hi_t = []
                new_carry_r = []
                new_carry_i = []
                for g in range(RT):
                    px = pxp.tile([128, CH], F32, name=f"px{g}", tag="px")
                    for d in range(DT):
                        nc.tensor.matmul(
                            px,
                            lhsT=wpt_t[d][:, g * 128:(g + 1) * 128],
                            rhs=uT[d],
                            start=(d == 0), stop=(d == DT - 1))
                    xtr = wp.tile([128, CH], F32, name=f"xtr{g}", tag=f"xtr{g}",
                                  bufs=1)
                    nc.vector.tensor_tensor(xtr, px, C_t[g], op=AX.mult)
                    xti = wp.tile([128, CH], F32, name=f"xti{g}", tag=f"xti{g}",
                                  bufs=1)
                    nc.vector.tensor_tensor(xti, px, S_t[g], op=AX.mult)

                    gr = wp.tile([128, CH], F32, name=f"gr{g}", tag=f"gr{g}", bufs=1)
                    nc.vector.tensor_tensor_scan(
                        gr, mag_t[g], xtr, carry_r[g], op0=AX.mult, op1=AX.add)
                    gi = wp.tile([128, CH], F32, name=f"gi{g}", tag=f"gi{g}", bufs=1)
                    nc.vector.tensor_tensor_scan(
                        gi, mag_t[g], xti, carry_i[g], op0=AX.mult, op1=AX.subtract)

                    ncr = sp.tile([128, 1], F32, name=f"ncr{g}", tag=f"ncr{g}",
                                  bufs=2)
                    nc.vector.tensor_copy(ncr, gr[:, CH - 1:CH])
                    new_carry_r.append(ncr)
                    nci = sp.tile([128, 1], F32, name=f"nci{g}", tag=f"nci{g}",
                                  bufs=2)
                    nc.vector.tensor_copy(nci, gi[:, CH - 1:CH])
                    new_carry_i.append(nci)

                    # h_r = C*g_r - S*g_i ; h_i = C*g_i + S*g_r
                    ta = wp.tile([128, CH], F32, name="ta", tag="ta", bufs=2)
                    nc.vector.tensor_tensor(ta, C_t[g], gr, op=AX.mult)
                    tb = wp.tile([128, CH], F32, name="tb", tag="tb", bufs=2)
                    nc.gpsimd.tensor_tensor(tb, S_t[g], gi, op=AX.mult)
                    hr = wp.tile([128, CH], F32, name=f"hr{g}", tag=f"hr{g}", bufs=1)
                    nc.vector.tensor_tensor(hr, ta, tb, op=AX.subtract)
                    tc2 = wp.tile([128, CH], F32, name="tc2", tag="tc2", bufs=2)
                    nc.gpsimd.tensor_tensor(tc2, C_t[g], gi, op=AX.mult)
                    td = wp.tile([128, CH], F32, name="td", tag="td", bufs=2)
                    nc.vector.tensor_tensor(td, S_t[g], gr, op=AX.mult)
                    hi = wp.tile([128, CH], F32, name=f"hi{g}", tag=f"hi{g}", bufs=1)
                    nc.gpsimd.tensor_tensor(hi, tc2, td, op=AX.add)
                    hr_t.append(hr)
                    hi_t.append(hi)

                carry_r = new_carry_r
                carry_i = new_carry_i

                # ---- residual matmul + h transpose + layernorm per subtile ----
                for s in range(SUB):
                    py = pyp.tile([128, 2 * R], F32, name="py", tag="py")
                    for d in range(DT):
                        nc.tensor.matmul(
                            py[:, 0:512],
                            lhsT=uT[d][:, s * 128:(s + 1) * 128],
                            rhs=wrt_t[d][:, 0:512],
                            start=(d == 0), stop=False)
                        nc.tensor.matmul(
                            py[:, 512:1024],
                            lhsT=uT[d][:, s * 128:(s + 1) * 128],
                            rhs=wrt_t[d][:, 512:1024],
                            start=(d == 0), stop=False)
                    for g in range(RT):
                        nc.tensor.matmul(
                            py[:, g * 128:(g + 1) * 128],
                            lhsT=hr_t[g][:, s * 128:(s + 1) * 128],
                            rhs=ident, is_transpose=True,
                            start=False, stop=True)
                        nc.tensor.matmul(
                            py[:, 512 + g * 128:512 + (g + 1) * 128],
                            lhsT=hi_t[g][:, s * 128:(s + 1) * 128],
                            rhs=ident, is_transpose=True,
                            start=False, stop=True)

                    # layernorm over the 1024 features
                    stats = sp.tile([128, 12], F32, name="stats", tag="stats")
                    nc.vector.bn_stats(stats[:, 0:6], py[:, 0:512])
                    nc.vector.bn_stats(stats[:, 6:12], py[:, 512:1024])
                    mv = sp.tile([128, 2], F32, name="mv", tag="mv")
                    nc.vector.bn_aggr(mv, stats)
                    varep = sp.tile([128, 1], F32, name="varep", tag="varep")
                    nc.vector.tensor_scalar_add(varep, mv[:, 1:2], LN_EPS)
                    rec = sp.tile([128, 1], F32, name="rec", tag="rec")
                    nc.vector.reciprocal(rec, varep)
                    rstd = sp.tile([128, 1], F32, name="rstd", tag="rstd")
                    nc.scalar.sqrt(rstd, rec)
                    nbias = sp.tile([128, 1], F32, name="nbias", tag="nbias")
                    nc.vector.scalar_tensor_tensor(
                        nbias, mv[:, 0:1], -1.0, rstd, op0=AX.mult, op1=AX.mult)
                    osb = wp.tile([128, 2 * R], F32, name="osb", tag="osb", bufs=2)
                    nc.scalar.activation(osb, py, AF.Identity, bias=nbias,
                                         scale=rstd)
                    if apply_gamma_beta:
                        nc.vector.tensor_tensor(osb, osb, gam, op=AX.mult)
                        nc.gpsimd.tensor_tensor(osb, osb, bet, op=AX.add)
                    r0 = c * CH + s * 128
                    nc.sync.dma_start(out=out_b[r0:r0 + 128, :], in_=osb)

                if c == NCH - 1:
                    hr_last = hr_t
                    hi_last = hi_t

            # ---- final state outputs ----
            for g in range(RT):
                nc.sync.dma_start(out=fin_r[g * 128:(g + 1) * 128, :],
                                  in_=hr_last[g][:, CH - 1:CH])
                nc.sync.dma_start(out=fin_i[g * 128:(g + 1) * 128, :],
                                  in_=hi_last[g][:, CH - 1:CH])

    nc.compile()
    return nc


def _prep_host(u, h0_r, h0_i, lambda_raw, omega, W_proj, W_res, ln_gamma, ln_beta):
    lam = -5.0 / (1.0 + np.exp(-lambda_raw.astype(np.float64)))
    mag = np.exp(lam).astype(np.float32)

    t_idx = np.arange(1, T + 1, dtype=np.float64)
    ang = omega.astype(np.float64)[:, None] * t_idx[None, :]
    ctab = np.cos(ang).astype(np.float32)
    stab = np.sin(ang).astype(np.float32)

    magb = np.ascontiguousarray(np.broadcast_to(mag[:, None], (R, CH)))
    wpt = np.ascontiguousarray(W_proj.T)
    wrt = np.ascontiguousarray(W_res.T)
    idn = np.eye(128, dtype=np.float32)
    gmb = np.ascontiguousarray(
        np.broadcast_to(ln_gamma[None, :], (128, 2 * R))).astype(np.float32)
    btb = np.ascontiguousarray(
        np.broadcast_to(ln_beta[None, :], (128, 2 * R))).astype(np.float32)

    shared = dict(wpt=wpt, wrt=wrt, ctab=ctab, stab=stab, magb=magb, idn=idn,
                  gmb=gmb, btb=btb)
    in_maps = []
    for b in range(B):
        m = dict(shared)
        m["u_b"] = np.ascontiguousarray(u[b])
        m["h0r"] = np.ascontiguousarray(h0_r[b][:, None])
        m["h0i"] = np.ascontiguousarray(h0_i[b][:, None])
        in_maps.append(m)
    return in_maps


def kernel(u, h0_r, h0_i, lambda_raw, omega, W_proj, W_res, ln_gamma, ln_beta,
           mm_dtype=None, trace=False):
    mm_dtype = F32R if mm_dtype is None else mm_dtype
    apply_gb = not (np.all(ln_gamma == 1.0) and np.all(ln_beta == 0.0))
    key = (str(mm_dtype), apply_gb)
    if key not in _CACHE:
        _CACHE[key] = build_program(mm_dtype=mm_dtype, apply_gamma_beta=apply_gb)
    nc = _CACHE[key]

    in_maps = _prep_host(u, h0_r, h0_i, lambda_raw, omega, W_proj, W_res,
                         ln_gamma, ln_beta)
    res = run_bass_kernel_spmd(nc, in_maps, core_ids=list(range(B)), trace=trace)

    out = np.stack([res.results[b]["out_b"] for b in range(B)])
    final_r = np.stack([res.results[b]["fin_r"][:, 0] for b in range(B)])
    final_i = np.stack([res.results[b]["fin_i"][:, 0] for b in range(B)])
    kernel.last_results = res
    return out, final_r, final_i
